# revision 35
# baseline (speedup 1.0000x reference)
"""DGI (3-layer GCN encoder x2 + bilinear discriminator) Trainium2 Bass kernel.

Strategy (8 NeuronCores, 1D row-parallel over nodes):
  - Each core owns a 2048-row block of the 16384-node graph.
  - adj is pre-transposed + scaled by 2^14 + cast to fp8e4m3 on the host, so
    each core receives adjT_block [16384 (cols), 2048 (rows)] fp8. The PE
    mixes fp8 adj with fp16 activations at full rate; fp8 halves the HBM
    traffic of the 3 adj passes. The 2^14 scale keeps adj entries (~1/n) in
    e4m3's normal range and is folded into existing copy/activation ops.
  - Layer 1 uses associativity: q1 = adj @ [X1|X2] (128-wide, 4x fewer MACs
    than adj @ [X1 W1|X2 W1]) in the "flipped" form with the seq chunk as the
    stationary operand and the adjT slab moving, so the PE emits q1 transposed
    [128 dims, rows] directly; h1T = relu(W1T q1T + b1) via tiny matmuls --
    no PE transposes at all for layer 1.
  - Activations p = (XW) are kept full (all 16384 nodes) in SBUF in fp16,
    both encoders packed side by side in the free dim. After layer 2, the
    local h block is PE-transposed, multiplied by the next W, and the local
    p_next block [2048, d'] is AllGather'ed across the 8 cores.
  - Layer 3 is flipped like layer 1 (p3 stationary, adjT slab moving).
  - Readout: node-sum of h3 (enc1) via DVE free-dim reduce on h3T, AllGather,
    sigmoid -> c; cw = wd @ c via tiny matmuls; scores via PE matvec on h3T.
"""

import sys
import time

import numpy as np

sys.path.insert(0, "/opt/trn_rl_repo")

import concourse.bass as bass  # noqa: E402
import concourse.mybir as mybir  # noqa: E402
import concourse.tile as tile  # noqa: E402
from concourse import bacc  # noqa: E402

P = 128
NCORES = 8
D0, D1, D2, D3 = 64, 264, 164, 64
NS1, NS2 = 3, 2  # 128-subtiles of the (padded) contraction dims 384, 256
SCALE = 16384.0
S2 = 4096.0  # extra p2 scale so fp8e4m3 sees ~[2^-4, 2^6] magnitudes
# p2 fp8 packing (PW2 wide): [enc1 d0:128 | enc2 d0:128 | enc1 d128:164,
# pad 28 | enc2 d128:164, pad 28] -- tails at partition bases 0 / 64 so the
# layer-2 psum -> h2T activations never shift partitions.
PW2 = 3 * P
DT8 = mybir.dt.float8e4
DT16 = mybir.dt.float16
DT32 = mybir.dt.float32
AF = mybir.ActivationFunctionType
ALU = mybir.AluOpType


def _params(n):
    R = n // NCORES
    RC = R // P
    KT = n // P
    GSZ = 4 if RC % 4 == 0 else (2 if RC % 2 == 0 else 1)  # row-chunks per m-group
    return dict(
        R=R,
        RC=RC,
        KT=KT,
        GSZ=GSZ,
        NG=RC // GSZ,
        KO=4 if KT % 4 == 0 else 1,  # k-tiles per slab DMA
        # k-tiles per resident p chunk: half a rank's row block, so the
        # p-AllGather can be split into two halves that overlap compute
        CH=max(1, R // (2 * P)),
        SEQW=min(2048, n),
        SCW=min(512, R),  # score output chunk
    )


def build_program(n=16384, sim=False, mock_coll=False):
    pr = _params(n)
    R, RC, KT, GSZ, NG, KO, CH, SEQW, SCW = (
        pr["R"],
        pr["RC"],
        pr["KT"],
        pr["GSZ"],
        pr["NG"],
        pr["KO"],
        pr["CH"],
        pr["SEQW"],
        pr["SCW"],
    )
    NPC = KT // CH

    nc = bacc.Bacc(
        "TRN2",
        target_bir_lowering=False,
        debug=False,
        num_devices=1 if sim else NCORES,
    )

    # adjT pre-tiled on host: [NG, KT//KO, 128, KO, GSZ*P] so each slab DMA is
    # a fully-contiguous block with 2KB-per-partition descriptors.
    A = nc.dram_tensor(
        "adjT", [NG, KT // KO, P, KO, GSZ * P], DT8, kind="ExternalInput"
    ).ap()
    # p1 = [seq1 | seq2] pre-chunked on host: [NPC, P, CH, 2*D0]
    P1T = nc.dram_tensor(
        "p1t", [NPC, P, CH, 2 * D0], DT16, kind="ExternalInput"
    ).ap()
    W1 = nc.dram_tensor("w1", [P, D1], DT16, kind="ExternalInput").ap()
    W2 = nc.dram_tensor("w2", [NS1 * P, D2], DT16, kind="ExternalInput").ap()
    W3 = nc.dram_tensor("w3", [NS2 * P, D3], DT16, kind="ExternalInput").ap()
    B1 = nc.dram_tensor("b1c", [P, NS1], DT32, kind="ExternalInput").ap()
    B2 = nc.dram_tensor("b2c", [P, NS2], DT32, kind="ExternalInput").ap()
    B3 = nc.dram_tensor("b3", [P, 1], DT32, kind="ExternalInput").ap()
    WDT = nc.dram_tensor("wdt", [P, D3], DT32, kind="ExternalInput").ap()
    SB = nc.dram_tensor("sb", [1, 2 * R], DT32, kind="ExternalInput").ap()
    OUT = nc.dram_tensor("out", [2, R], DT32, kind="ExternalOutput").ap()

    rg = [list(range(NCORES))]
    no_coll = sim or mock_coll
    shared_kw = {} if no_coll else {"addr_space": "Shared"}

    def ag(src, dst, nrows):
        """AllGather src -> dst; in sim mode model only the local shard DMA."""
        if no_coll:
            nc.gpsimd.dma_start(dst[:][0:nrows, :], src[:])
        else:
            nc.gpsimd.collective_compute(
                "AllGather",
                ALU.bypass,
                replica_groups=rg,
                ins=[src.opt()],
                outs=[dst.opt()],
            )

    with tile.TileContext(nc) as tc:
        with (
            tc.tile_pool(name="const", bufs=1) as cp,
            tc.tile_pool(name="p", bufs=NPC) as pp,
            tc.tile_pool(name="slab", bufs=3) as slp,
            tc.tile_pool(name="h", bufs=4) as hp,
            tc.tile_pool(name="hT", bufs=2) as htp,
            tc.tile_pool(name="misc", bufs=4) as mp_,
            tc.tile_pool(name="sc", bufs=6) as scp_,
            tc.tile_pool(name="ploc", bufs=2) as plp,
            tc.tile_pool(name="ps", bufs=8, space="PSUM") as ps,
            tc.tile_pool(name="dram", bufs=1, space="DRAM") as dram,
        ):
            # ---- constants -> SBUF
            w1t = cp.tile([P, D1], DT16, name="w1t")
            nc.sync.dma_start(w1t[:], W1[:])
            w2t = cp.tile([P, NS1, D2], DT16, name="w2t")
            nc.sync.dma_start(w2t[:], W2.rearrange("(s p) d -> p s d", p=P))
            w3t = cp.tile([P, NS2, D3], DT16, name="w3t")
            nc.sync.dma_start(w3t[:], W3.rearrange("(s p) d -> p s d", p=P))
            b1c = cp.tile([P, NS1], DT32, name="b1c")
            nc.sync.dma_start(b1c[:], B1[:])
            b2c = cp.tile([P, NS2], DT32, name="b2c")
            nc.sync.dma_start(b2c[:], B2[:])
            b3t = cp.tile([P, 1], DT32, name="b3t")
            nc.sync.dma_start(b3t[:], B3[:])
            wdtt = cp.tile([P, D3], DT32, name="wdtt")
            nc.sync.dma_start(wdtt[:], WDT[:])

            # ---- DRAM bounce buffers for collectives (split in row-halves so
            # each AllGather overlaps the next compute phase)
            RH = R // 2
            p2l = [dram.tile([RH, PW2], DT8, name=f"p2l{h}") for h in range(2)]
            p2f = [
                dram.tile([n // 2, PW2], DT8, name=f"p2f{h}", **shared_kw)
                for h in range(2)
            ]
            p3l = [dram.tile([RH, 2 * D3], DT16, name=f"p3l{h}") for h in range(2)]
            p3f = [
                dram.tile([n // 2, 2 * D3], DT16, name=f"p3f{h}", **shared_kw)
                for h in range(2)
            ]
            ssi = dram.tile([64, 1], DT32, name="ssi")
            ssg = dram.tile([64 * NCORES, 1], DT32, name="ssg", **shared_kw)

            # ---- p1 chunks: [seq1 | seq2] loaded straight from DRAM (no
            # matmul -- layer 1 computes adj @ X first, W1 applied after)
            pch = [
                pp.tile([P, CH, 2 * D0], DT16, tag="p", name=f"p1c{c}")
                for c in range(NPC)
            ]
            for c in range(NPC):
                eng = (nc.sync, nc.scalar)[c % 2]
                eng.dma_start(pch[c][:], P1T[c])

            # kb visit order: first-half chunks (even) before second-half, so
            # a layer can start while the second AllGather half is in flight.
            # Only valid when each slab stays within one chunk half.
            if KO <= CH:
                kb_order = [j for j in range(KT // KO) if ((j * KO) // CH) % 2 == 0]
                kb_order += [j for j in range(KT // KO) if ((j * KO) // CH) % 2 == 1]
            else:
                kb_order = list(range(KT // KO))

            # ---- Layer 1 target: hT1[e] = (2^14 * relu(adj @ seq_e @ W1 + b1)).T
            hT1 = [
                htp.tile([P, NS1, R], DT16, tag="hT", name=f"h1T{e}") for e in range(2)
            ]
            for e in range(2):
                # zero the partial last k-subtile (h1T writes only rows
                # 0:(D1-2P) of it); start-partition slicing must be
                # 32-aligned, so zero the whole [P, R] slice.
                nc.vector.memset(hT1[e][:, NS1 - 1, :], 0.0)

            # ---- p_next = h @ W (local rows), staged per row-half: the first
            # half's matmuls + AllGather are issued mid-layer (as soon as the
            # hT rows exist), the loads at layer end in consumption order.
            def p_stage_half(
                hT, wt, ns, d_next, width, packs, pscale, pdt, inbase,
                ploc_bufs, pf_bufs, tagix, h,
            ):
                """Compute the local rows' p_next = h @ W_next, pack into the
                next layer's fp16/fp8 chunk column layout, and AllGather.

                packs(e) -> [(src_lo, src_hi, dst_off)] column copies;
                inbase(e, ds) -> (base, pwidth) partition window of hT/wt.
                """
                RC2 = RC // 2
                ploc = plp.tile(
                    [P, RC2, width], pdt, tag="ploc", name=f"pl{tagix}_{h}"
                )
                if width != 2 * d_next:
                    nc.vector.memset(ploc[:], 0.0)
                for rcl in range(RC2):
                    rc = h * RC2 + rcl
                    for e in range(2):
                        pq = ps.tile(
                            [P, d_next], DT32, tag="ps", name=f"pq{tagix}_{e}_{rc}"
                        )
                        for ds in range(ns):
                            b, pw = inbase(e, ds)
                            nc.tensor.matmul(
                                pq[:],
                                hT[e][b : b + pw, ds, rc * P : (rc + 1) * P],
                                wt[b : b + pw, ds, :],
                                start=(ds == 0),
                                stop=(ds == ns - 1),
                            )
                        for lo, hi, off in packs(e):
                            nc.scalar.mul(
                                ploc[:, rcl, off : off + hi - lo],
                                pq[:, lo:hi],
                                pscale,
                            )
                nc.sync.dma_start(
                    ploc_bufs[h][:].rearrange("(rc p) d -> p rc d", p=P), ploc[:]
                )
                ag(ploc_bufs[h], pf_bufs[h], RH)

            def make_pnext(width, tagix, pdt):
                # chunk c covers k-tiles [c*CH, (c+1)*CH) = rank c//2, half c%2
                return [
                    pp.tile([P, CH, width], pdt, tag="p", name=f"p{tagix}c{c}")
                    for c in range(NPC)
                ]

            def p_loads_half(newp, pf_bufs, h):
                # gpsimd (SWDGE) ring so a slot-wait here never stalls the
                # sync/scalar rings that stream adjT slabs
                RH_ = CH * P  # rows per (rank, half)
                for c in range(h, NPC, 2):
                    rank = c // 2
                    nc.gpsimd.dma_start(
                        newp[c][:],
                        pf_bufs[h][:][rank * RH_ : (rank + 1) * RH_, :].rearrange(
                            "(ko p) d -> p ko d", p=P
                        ),
                    )

            p2c = make_pnext(PW2, 2, DT8)
            packs2 = lambda e: [(0, P, e * P), (P, D2, 2 * P + e * 64)]
            base2 = lambda e, ds: (0, P)

            def stage2(h):
                p_stage_half(
                    hT1, w2t, NS1, D2, PW2, packs2, S2 / SCALE, DT8, base2,
                    p2l, p2f, 2, h,
                )
                p_loads_half(p2c, p2f, h)

            # ---- Layer 1, flipped: p1 (seq, [128, 2*D0]) is the stationary
            # operand, the adjT slab the moving one, so the PE emits
            # q1T = 2^14 * ([X1|X2].T adj.T) [128 dims, 512 rows] directly.
            # Then h1T[e] = relu(W1.T q1T[e] + 2^14 b1) via tiny matmuls.
            for g in range(NG):
                q1 = ps.tile([P, GSZ * P], DT32, tag="ps", name=f"q1_{g}")
                for kb in range(KT // KO):
                    slab = slp.tile(
                        [P, KO, GSZ * P], DT8, tag="slab", name=f"sl1_{g}_{kb}"
                    )
                    eng = nc.sync if kb % 2 == 0 else nc.scalar
                    eng.dma_start(slab[:], A[g, kb])
                    for ko in range(KO):
                        k = kb * KO + ko
                        nc.tensor.matmul(
                            q1[:],
                            pch[k // CH][:, k % CH, :],
                            slab[:, ko, :],
                            start=(kb == 0 and ko == 0),
                            stop=(kb == KT // KO - 1 and ko == KO - 1),
                        )
                q1s = hp.tile([P, GSZ * P], DT16, tag="h", name=f"q1s_{g}")
                nc.vector.tensor_copy(q1s[:], q1[:])
                for e in range(2):
                    for ds in range(NS1):
                        csz = min(P, D1 - ds * P)
                        hps = ps.tile(
                            [csz, GSZ * P], DT32, tag="ps", name=f"h1p_{g}_{e}_{ds}"
                        )
                        nc.tensor.matmul(
                            hps[:],
                            w1t[e * D0 : (e + 1) * D0, ds * P : ds * P + csz],
                            q1s[e * D0 : (e + 1) * D0, :],
                            start=True,
                            stop=True,
                        )
                        nc.scalar.activation(
                            hT1[e][0:csz, ds, g * GSZ * P : (g + 1) * GSZ * P],
                            hps[:],
                            AF.Relu,
                            bias=b1c[0:csz, ds : ds + 1],
                        )
                if g == NG // 2 - 1:
                    stage2(0)
            if NG < 2:
                stage2(0)
            stage2(1)
            pch = p2c

            # ---- Layer 2, flipped + fp8 DoubleRow: p2 (fp8, S2-scaled) is the
            # stationary operand in 3 column-splits of PW2, the adjT slab pair
            # the moving one -- two k-tiles per matmul at 2 rows/cycle. The
            # psum splits come out as h2T dims: s0 = enc1 d0:128, s1 = enc2
            # d0:128, s2 = enc1 tail @ partitions 0:36 + enc2 tail @ 64:100.
            hT2 = [
                htp.tile([P, NS2, R], DT16, tag="hT", name=f"h2T{e}") for e in range(2)
            ]
            for e in range(2):
                nc.vector.memset(hT2[e][:, NS2 - 1, :], 0.0)

            p3c = make_pnext(2 * D3, 3, DT16)
            packs3 = lambda e: [(0, D3, e * D3)]
            base3 = lambda e, ds: (0, P) if ds == 0 else (e * 64, 64)

            def stage3(h):
                p_stage_half(
                    hT2, w3t, NS2, D3, 2 * D3, packs3, 1.0 / SCALE, DT16, base3,
                    p3l, p3f, 3, h,
                )
                p_loads_half(p3c, p3f, h)

            DR = mybir.MatmulPerfMode.DoubleRow
            for g in range(NG):
                qs = [
                    ps.tile([P, GSZ * P], DT32, tag="ps", name=f"q2_{g}_{s}")
                    for s in range(3)
                ]
                for ki, kb in enumerate(kb_order):
                    slab = slp.tile(
                        [P, KO, GSZ * P], DT8, tag="slab", name=f"sl2_{g}_{kb}"
                    )
                    eng = nc.sync if kb % 2 == 0 else nc.scalar
                    eng.dma_start(slab[:], A[g, kb])
                    for jp in range(KO // 2):
                        k = kb * KO + 2 * jp
                        c, m = k // CH, k % CH
                        for s in range(3):
                            nc.tensor.matmul(
                                qs[s][:],
                                pch[c][:, m : m + 2, s * P : (s + 1) * P],
                                slab[:, 2 * jp : 2 * jp + 2, :],
                                start=(ki == 0 and jp == 0),
                                stop=(ki == len(kb_order) - 1 and jp == KO // 2 - 1),
                                perf_mode=DR,
                            )
                cols = slice(g * GSZ * P, (g + 1) * GSZ * P)
                for e in range(2):
                    nc.scalar.activation(
                        hT2[e][:, 0, cols],
                        qs[e][:],
                        AF.Relu,
                        bias=b2c[:, 0:1],
                        scale=1.0 / S2,
                    )
                    nc.scalar.activation(
                        hT2[e][e * 64 : e * 64 + 36, 1, cols],
                        qs[2][e * 64 : e * 64 + 36, :],
                        AF.Relu,
                        bias=b2c[e * 64 : e * 64 + 36, 1:2],
                        scale=1.0 / S2,
                    )
                if g == NG // 2 - 1:
                    stage3(0)
            if NG < 2:
                stage3(0)
            stage3(1)
            pch = p3c

            # ---- Layer 3, flipped: p3[k] is the stationary operand, the adjT
            # slab the moving one, so the PE emits q3 transposed directly:
            # psum[2*D3 dims, 512 rows]. Partitions 0:64 are enc1 dims, 64:128
            # enc2. One N=512 matmul per k-tile, no PE transposes, bias+relu
            # as a single per-partition-bias activation.
            h3T = htp.tile([P, R], DT16, tag="hT", name="h3Tcat")
            for g in range(NG):
                q3 = ps.tile([P, GSZ * P], DT32, tag="ps", name=f"q3_{g}")
                for ki, kb in enumerate(kb_order):
                    slab = slp.tile(
                        [P, KO, GSZ * P], DT8, tag="slab", name=f"sl3_{g}_{kb}"
                    )
                    eng = (nc.sync, nc.scalar, nc.gpsimd)[kb % 3]
                    eng.dma_start(slab[:], A[g, kb])
                    for ko in range(KO):
                        k = kb * KO + ko
                        nc.tensor.matmul(
                            q3[:],
                            pch[k // CH][:, k % CH, :],
                            slab[:, ko, :],
                            start=(ki == 0 and ko == 0),
                            stop=(ki == len(kb_order) - 1 and ko == KO - 1),
                        )
                nc.scalar.activation(
                    h3T[:, g * GSZ * P : (g + 1) * GSZ * P],
                    q3[:],
                    AF.Relu,
                    bias=b3t[:],
                )

            # ---- readout: c = sigmoid(mean_n h3_enc1); cw = wd @ c; sc = h3 @ cw
            # node-sum via AllGather + local reduce (AG floor is ~2x lower
            # than AllReduce's)
            ss = mp_.tile([P, 1], DT32, tag="misc", name="ss")
            nc.vector.reduce_sum(
                ss[0:64, :], h3T[0:64, :], axis=mybir.AxisListType.X
            )
            nc.sync.dma_start(ssi[:], ss[0:64, :])
            ag(ssi, ssg, 64)
            cin = mp_.tile([64, NCORES], DT32, tag="misc", name="cin")
            nc.sync.dma_start(
                cin[:], ssg[:].rearrange("(c p) one -> p (c one)", p=64)
            )
            cin2 = mp_.tile([64, 1], DT32, tag="misc", name="cin2")
            nc.vector.reduce_sum(cin2[:], cin[:], axis=mybir.AxisListType.X)
            ccol = mp_.tile([P, 1], DT32, tag="misc", name="ccol")
            nc.vector.memset(ccol[:], 0.0)
            nc.scalar.activation(
                ccol[0:64, :], cin2[:], AF.Sigmoid, scale=1.0 / (SCALE * n)
            )
            cwps = ps.tile([64, 1], DT32, tag="ps", name="cwps")
            nc.tensor.matmul(cwps[:], wdtt[:], ccol[:], start=True, stop=True)
            # two masked copies of cw: cwa selects enc1 partitions, cwb enc2
            cw16 = [
                mp_.tile([P, 1], DT16, tag="misc", name=f"cw16_{e}") for e in range(2)
            ]
            for e in range(2):
                nc.vector.memset(cw16[e][:], 0.0)
                nc.vector.tensor_copy(cw16[e][e * D3 : (e + 1) * D3, :], cwps[:])
            # score epilogue: all matmuls issued back-to-back, per-chunk
            # scale/bias/store pipelined on dedicated pool slots
            scps = []
            for e in range(2):
                for j in range(R // SCW):
                    scp = ps.tile([1, SCW], DT32, tag="ps", name=f"scp{e}_{j}")
                    nc.tensor.matmul(
                        scp[:],
                        cw16[e][:],
                        h3T[:, j * SCW : (j + 1) * SCW],
                        start=True,
                        stop=True,
                    )
                    scps.append((e, j, scp))
                    sbc = scp_.tile([1, SCW], DT32, tag="sc", name=f"sbc{e}_{j}")
                    nc.sync.dma_start(
                        sbc[:], SB[:, e * R + j * SCW : e * R + (j + 1) * SCW]
                    )
                    sct = scp_.tile([1, SCW], DT32, tag="sc", name=f"sct{e}_{j}")
                    nc.scalar.mul(sct[:], scp[:], 1.0 / SCALE)
                    ot = scp_.tile([1, SCW], DT32, tag="sc", name=f"ot{e}_{j}")
                    nc.vector.tensor_tensor(ot[:], sct[:], sbc[:], ALU.add)
                    nc.scalar.dma_start(OUT[e : e + 1, j * SCW : (j + 1) * SCW], ot[:])

    nc.compile()
    return nc


# ---------------------------------------------------------------------------
# host-side input prep


def _blocked_transpose(a):
    n = a.shape[0]
    out = np.empty((a.shape[1], n), a.dtype)
    B = 512
    for i in range(0, n, B):
        for j in range(0, a.shape[1], B):
            out[j : j + B, i : i + B] = a[i : i + B, j : j + B].T
    return out


def prep_concat_inputs(inputs, n):
    R = n // NCORES
    adj = np.asarray(inputs["adj"], np.float32)[0]
    seq1 = np.asarray(inputs["seq1"], np.float32)[0]
    seq2 = np.asarray(inputs["seq2"], np.float32)[0]
    w1 = np.asarray(inputs["w1"], np.float32)
    w2 = np.asarray(inputs["w2"], np.float32)
    w3 = np.asarray(inputs["w3"], np.float32)
    b1 = np.asarray(inputs["b1"], np.float32)
    b2 = np.asarray(inputs["b2"], np.float32)
    b3 = np.asarray(inputs["b3"], np.float32)
    wd = np.asarray(inputs["wd"], np.float32)
    bd = np.float32(np.asarray(inputs["bd"]))
    sb1 = np.asarray(inputs["samp_bias1"], np.float32)[0]
    sb2 = np.asarray(inputs["samp_bias2"], np.float32)[0]

    pr = _params(n)
    KT, KO, GSZ, NG, CH = pr["KT"], pr["KO"], pr["GSZ"], pr["NG"], pr["CH"]
    KB, W = KT // KO, GSZ * P
    NPC = KT // CH

    np8 = mybir.dt.np(DT8)
    a8 = (adj * np.float32(SCALE)).astype(np8)
    a8T = _blocked_transpose(a8)  # [n, n]; a8T[c, r] = scaled adj[r, c]
    del a8
    # per-core block [n, R] -> slab-tiled [NG, KB, P, KO, W] (contiguous slabs)
    adjT_cat = np.empty((NCORES * NG, KB, P, KO, W), np8)
    for c in range(NCORES):
        blk = np.ascontiguousarray(a8T[:, c * R : (c + 1) * R])
        t = blk.reshape(KB, KO, P, NG, W).transpose(3, 0, 2, 1, 4)
        adjT_cat[c * NG : (c + 1) * NG] = t
    del a8T

    def padz(a, shape):
        out = np.zeros(shape, np.float16)
        out[: a.shape[0], : a.shape[1]] = a
        return out

    def rep(x):
        return np.tile(np.asarray(x), (NCORES, 1))

    # p1 chunks: [n, 2*D0] = [seq1 | seq2] -> [NPC, P, CH, 2*D0]
    p1 = np.concatenate([seq1, seq2], axis=1).astype(np.float16)
    p1t = np.ascontiguousarray(
        p1.reshape(NPC, CH, P, 2 * D0).transpose(0, 2, 1, 3)
    )

    b1col = np.zeros((P, NS1), np.float32)
    for ds in range(NS1):
        csz = min(P, D1 - ds * P)
        b1col[:csz, ds] = b1[ds * P : ds * P + csz] * SCALE

    # layer-2 per-partition bias columns for the flipped psum splits:
    # col 0 = b2[0:128] (enc1 & enc2 main), col 1 = b2 tail at partition
    # bases 0 (enc1) and 64 (enc2)
    b2col = np.zeros((P, NS2), np.float32)
    b2col[:, 0] = b2[0:P] * SCALE
    b2col[0 : D2 - P, 1] = b2[P:D2] * SCALE
    b2col[64 : 64 + D2 - P, 1] = b2[P:D2] * SCALE

    # w3 with the tail block duplicated at partition bases 0 and 64 of the
    # second 128-row plane (matches hT2's per-encoder tail placement)
    w3p = np.zeros((NS2 * P, D3), np.float16)
    w3p[0:P] = w3[0:P]
    w3p[P : P + D2 - P] = w3[P:D2]
    w3p[P + 64 : P + 64 + D2 - P] = w3[P:D2]

    cat = {
        "adjT": adjT_cat,
        "p1t": np.tile(p1t.reshape(1, -1), (NCORES, 1)).reshape(
            (NCORES * NPC, P, CH, 2 * D0)
        ),
        # w1 stacked twice along partitions so both encoders' q1 slices
        # (base partition 0 and 64) see a matching lhsT base partition
        "w1": rep(np.concatenate([w1, w1], axis=0).astype(np.float16)),
        "w2": rep(padz(w2, (NS1 * P, D2))),
        "w3": rep(w3p),
        "b1c": rep(b1col),
        "b2c": rep(b2col),
        "b3": rep(
            np.concatenate([b3, b3]).astype(np.float32)[:, None] * np.float32(SCALE)
        ),
        "wdt": rep(padz(wd.T, (P, D3)).astype(np.float32)),
        "sb": np.concatenate(
            [
                np.concatenate(
                    [sb1[c * R : (c + 1) * R] + bd, sb2[c * R : (c + 1) * R] + bd]
                )[None, :]
                for c in range(NCORES)
            ],
            axis=0,
        ).astype(np.float32),
    }
    return cat


# ---------------------------------------------------------------------------
# cached PJRT executor (compile once, run many)

_EXEC = {}


def make_state(nc):
    """Build a cached shard_map executable for a compiled Bass program."""
    import jax
    from jax.sharding import Mesh, NamedSharding, PartitionSpec
    from concourse import bass2jax as b2j

    b2j.install_neuronx_cc_hook()

    partition_name = (
        nc.partition_id_tensor.name if nc.partition_id_tensor else None
    )
    in_names = []
    out_names = []
    out_avals = []
    for alloc in nc.m.functions[0].allocations:
        if not isinstance(alloc, mybir.MemoryLocationSet):
            continue
        name = alloc.memorylocations[0].name
        if alloc.kind == "ExternalInput":
            if name != partition_name:
                in_names.append(name)
        elif alloc.kind == "ExternalOutput":
            out_names.append(name)
            out_avals.append(
                jax.core.ShapedArray(
                    tuple(alloc.tensor_shape), mybir.dt.np(alloc.dtype)
                )
            )
    n_params = len(in_names)
    all_names = in_names + out_names
    if partition_name is not None:
        all_names = all_names + [partition_name]

    def _body(*args):
        operands = list(args)
        if partition_name is not None:
            operands.append(b2j.partition_id_tensor())
        outs = b2j._bass_exec_p.bind(
            *operands,
            out_avals=tuple(out_avals),
            in_names=tuple(all_names),
            out_names=tuple(out_names),
            lowering_input_output_aliases=(),
            sim_require_finite=True,
            sim_require_nnan=True,
            nc=nc,
        )
        return tuple(outs)

    devices = jax.devices()[:NCORES]
    mesh = Mesh(np.asarray(devices), ("core",))
    spec = PartitionSpec("core")
    n_outs = len(out_names)
    sharded = jax.jit(
        b2j.shard_map(
            _body,
            mesh=mesh,
            in_specs=(spec,) * (n_params + n_outs),
            out_specs=(spec,) * n_outs,
            check_rep=False,
        ),
        keep_unused=True,
    )
    return {
        "nc": nc,
        "fn": sharded,
        "in_names": in_names,
        "out_names": out_names,
        "out_avals": out_avals,
        "mesh": mesh,
        "sharding": NamedSharding(mesh, spec),
        "dev_inputs": None,
        "dev_zouts": None,
    }


def _get_exec(n):
    if n in _EXEC:
        return _EXEC[n]
    state = make_state(build_program(n))
    _EXEC[n] = state
    return state


def _zero_outs(state):
    return [
        np.zeros((NCORES * a.shape[0], *a.shape[1:]), a.dtype)
        for a in state["out_avals"]
    ]


def _execute(state, cat_inputs=None):
    import jax

    if cat_inputs is not None:
        state["dev_inputs"] = [
            jax.device_put(cat_inputs[name], state["sharding"])
            for name in state["in_names"]
        ]
    if state["dev_zouts"] is None:
        state["dev_zouts"] = [
            jax.device_put(z, state["sharding"]) for z in _zero_outs(state)
        ]
    outs = state["fn"](*state["dev_inputs"], *state["dev_zouts"])
    return [np.asarray(o) for o in outs]


def kernel(**inputs):
    n = int(np.asarray(inputs["adj"]).shape[1])
    state = _get_exec(n)
    cat = prep_concat_inputs(inputs, n)
    outs = _execute(state, cat)
    # out tensor: [NCORES*2, R] -> per-core [2, R]
    R = n // NCORES
    o = outs[0].reshape(NCORES, 2, R)
    full = np.empty((1, 2 * n), np.float32)
    for c in range(NCORES):
        full[0, c * R : (c + 1) * R] = o[c, 0]
        full[0, n + c * R : n + (c + 1) * R] = o[c, 1]
    return full


def _run_chain(state, n_iters):
    """Enqueue n_iters executions back-to-back, block once at the end.
    The axon tunnel pipelines async dispatches, so the per-iteration
    marginal time approaches the on-device execution time."""
    t0 = time.perf_counter()
    outs = None
    for _ in range(n_iters):
        outs = state["fn"](*state["dev_inputs"], *state["dev_zouts"])
    for o in outs:
        o.block_until_ready()
    return time.perf_counter() - t0


def bench(n=16384, iters=10, reps=4, n_lo=4, n_hi=24):
    """Per-run device-execution time via two-point pipelined timing.

    Executions are enqueued without intermediate blocking; the fixed
    tunnel round-trip cancels in the (T(n_hi) - T(n_lo)) / (n_hi - n_lo)
    slope, leaving the marginal per-execution time."""
    state = _EXEC.get(n)
    assert state is not None and state["dev_inputs"] is not None, (
        "call kernel() first"
    )
    _run_chain(state, 2)  # warm
    slopes = []
    for _ in range(reps):
        t_lo = _run_chain(state, n_lo)
        t_hi = _run_chain(state, n_hi)
        slopes.append((t_hi - t_lo) / (n_hi - n_lo))
    return min(slopes), slopes

